# revision 27
# baseline (speedup 1.0000x reference)
"""Multi-head causal attention (B=2,S=2048,D=1024,H=16,DH=64) on 8 TRN2 cores.

Sharding: 2 heads per core (tensor parallel). Each core computes QKV for its
2 heads from the full x, causal attention, and its partial of the output
projection [B,S,D]. The host sums the 8 partials (the W_O head-sum).

On-device layouts (matmul contracts over the partition dim):
  QT/KT  [2*DH=128 part, S]   (heads stacked on partitions; 1/sqrt(DH) folded into W_Q)
  V      [S part (128-blocks), heads, DH+1]  (ones column -> softmax row-sums for free)
  S^T    [k 128 part, q 512]  per (k-block, q-tile); above-diagonal blocks skipped
  Z'^T   [DH+1 part, q 512]   accumulated over k-blocks; row DH = exp row-sum
  out    partial [B,S,D] bf16, summed across cores on host

Key scheduling ideas:
  - The two heads' score matmuls are emitted back-to-back with explicit
    tile_position (0,0)/(64,0) so they run CONCURRENTLY in the two 64-row
    strips of the PE array (K=DH=64 each).
  - Causal mask is applied as a 0/1 bf16 multiply on the exp output (DVE,
    cheap) instead of a -1e5 fp32 add on the scores PSUM.
  - All projection / output work is cut into small "filler pieces" that are
    interleaved at per-stage granularity into the attention pipeline; the
    tensor queue is FIFO, so PE work must sit between a score matmul and the
    exp-dependent AV matmul to cover the softmax latency.
  - x^T is DMAed s-tile-major across 4 queues so attention can start early;
    out DMA rotates queues so the tail drains while compute continues.
"""

import os
import sys
from collections import deque

import numpy as np

if "/opt/trn_rl_repo" not in sys.path:
    sys.path.insert(0, "/opt/trn_rl_repo")

import ml_dtypes

B, S, D, H, DH = 2, 2048, 1024, 16, 64
NCORES = 8
HPC = H // NCORES          # heads per core
P = 128
QT_W = 512                 # q-tile width
NQT = S // QT_W            # 4 q-tiles
NKB = S // P               # 16 k-blocks
NDC = D // P               # 8 contraction chunks for projections

BF16 = ml_dtypes.bfloat16

_CACHE = {}


def _build_nc(B=B, S=S, D=D, HPC=HPC, DH=DH):
    import concourse.tile as tile
    import concourse.mybir as mybir
    from concourse import bacc
    from contextlib import ExitStack

    QT_W = 512
    NQT = S // QT_W
    NKB = S // P
    NDC = D // P

    f32 = mybir.dt.float32
    bf16 = mybir.dt.bfloat16
    AF = mybir.ActivationFunctionType
    ALU = mybir.AluOpType

    nc = bacc.Bacc("TRN2", target_bir_lowering=False, debug=False,
                   num_devices=NCORES)

    xT = nc.dram_tensor("xT", [B, D, S], bf16, kind="ExternalInput").ap()
    wq_d = nc.dram_tensor("wq", [D, HPC * DH], bf16, kind="ExternalInput").ap()
    wk_d = nc.dram_tensor("wk", [D, HPC * DH], bf16, kind="ExternalInput").ap()
    wv_d = nc.dram_tensor("wv", [D, HPC * DH], bf16, kind="ExternalInput").ap()
    wo_d = nc.dram_tensor("wo", [HPC * DH, D], bf16, kind="ExternalInput").ap()
    bq_d = nc.dram_tensor("bq", [HPC * DH, 1], f32, kind="ExternalInput").ap()
    bk_d = nc.dram_tensor("bk", [HPC * DH, 1], f32, kind="ExternalInput").ap()
    msk_d = nc.dram_tensor("msk", [P, P], bf16, kind="ExternalInput").ap()
    out_d = nc.dram_tensor("out", [B, S, D], bf16, kind="ExternalOutput").ap()

    with tile.TileContext(nc) as tc, ExitStack() as ctx:
        const = ctx.enter_context(tc.tile_pool(name="const", bufs=1))
        qk_pool = ctx.enter_context(tc.tile_pool(name="qk", bufs=4))
        v_pool = ctx.enter_context(tc.tile_pool(name="v", bufs=2))
        pt_pool = ctx.enter_context(tc.tile_pool(name="pt", bufs=8))
        sm_pool = ctx.enter_context(tc.tile_pool(name="sm", bufs=4))
        zt_pool = ctx.enter_context(tc.tile_pool(name="zt", bufs=4))
        o_pool = ctx.enter_context(tc.tile_pool(name="o", bufs=3))
        # PSUM: 4 (scores, shared with rowsum-bcast) + 2 (Z' per head) + 2 (proj)
        st_ps = ctx.enter_context(tc.tile_pool(name="stps", bufs=4, space="PSUM"))
        z_ps = ctx.enter_context(tc.tile_pool(name="zps", bufs=2, space="PSUM"))
        mm_ps = ctx.enter_context(tc.tile_pool(name="mmps", bufs=2, space="PSUM"))

        dmaq = [nc.sync, nc.gpsimd, nc.scalar]

        # ---- resident constants ----
        # weights first (one per queue, in parallel), then x^T s-tile-major in
        # per-dc chunks so attention on tile 0 can start as early as possible.
        # Per-queue DMA sustains only ~90 GB/s, so what lands first is decided
        # by per-queue byte order, not emission order.
        wq_sb = const.tile([P, NDC, HPC * DH], bf16)
        nc.sync.dma_start(wq_sb[:], wq_d.rearrange("(dc p) m -> p dc m", p=P))
        bq_sb = const.tile([HPC * DH, 1], f32)
        nc.sync.dma_start(bq_sb[:], bq_d[:])
        wk_sb = const.tile([P, NDC, HPC * DH], bf16)
        nc.gpsimd.dma_start(wk_sb[:], wk_d.rearrange("(dc p) m -> p dc m", p=P))
        bk_sb = const.tile([HPC * DH, 1], f32)
        nc.gpsimd.dma_start(bk_sb[:], bk_d[:])
        wv_sb = const.tile([P, NDC, HPC * DH], bf16)
        nc.scalar.dma_start(wv_sb[:], wv_d.rearrange("(dc p) m -> p dc m", p=P))
        msk_sb = const.tile([P, P], bf16)
        nc.scalar.dma_start(msk_sb[:], msk_d[:])
        # rowsum broadcast selector: row DH (the exp row-sum row of the
        # copied-out Z' tile) maps to all 64 output partitions of one head.
        sel_sb = const.tile([DH + 1, DH], bf16)
        nc.vector.memset(sel_sb[:], 0.0)
        nc.vector.memset(sel_sb[DH:DH + 1, :], 1.0)

        # The scalar queue must stay clear of bulk DMA: exp runs there, and a
        # dma_start blocked on ring credits would stall the whole softmax
        # pipeline. Scalar only issues for batch-0 s-tiles 0/1 (early, before
        # any exp); everything later goes to sync/gpsimd.
        xt_sb = const.tile([P, B, NDC, S], bf16)
        wo_sb = const.tile([HPC * DH, D], bf16)
        qrot = 0
        for b in range(B):
            for st in range(NQT):
                s0, s1 = st * QT_W, (st + 1) * QT_W
                nq = 3 if (b == 0 and st < 2) else 2
                for dc in range(NDC):
                    dmaq[qrot % nq].dma_start(
                        xt_sb[:, b, dc, s0:s1],
                        xT[b, dc * P:(dc + 1) * P, s0:s1])
                    qrot += 1
            if b == 0:
                nc.sync.dma_start(wo_sb[:], wo_d[:])

        qt = {}
        kt = {}
        vv = {}
        for b in range(B):
            qt[b] = qk_pool.tile([P, S], bf16, tag="qt", name=f"qt{b}")
            kt[b] = qk_pool.tile([P, S], bf16, tag="qt", name=f"kt{b}")
            vv[b] = v_pool.tile([P, NKB, HPC, DH + 1], bf16, tag="v",
                                name=f"v{b}")
            nc.vector.memset(vv[b][:, :, :, DH:DH + 1], 1.0)

        # ---------- filler pieces ----------
        # Small closures, each ~1-4 matmuls, interleaved between attention
        # pipeline stages. The deque order respects data dependencies; the
        # `due` tag forces a drain before the unit that needs the results.
        backlog = deque()

        def qk_pieces(b, t):
            """Q and K projection for q-tile t of batch b: 4 pieces."""
            out = []
            for w_sb, bias, dst in ((wq_sb, bq_sb, qt[b]),
                                    (wk_sb, bk_sb, kt[b])):
                hold = {}

                def p1(w_sb=w_sb, hold=hold):
                    ps = mm_ps.tile([P, QT_W], f32, tag="mm", name="qkps")
                    for dc in range(4):
                        nc.tensor.matmul(
                            ps[:], w_sb[:, dc, :],
                            xt_sb[:, b, dc, t * QT_W:(t + 1) * QT_W],
                            start=(dc == 0), stop=False,
                            skip_group_check=True)
                    hold[0] = ps

                def p2(w_sb=w_sb, bias=bias, dst=dst, hold=hold):
                    ps = hold[0]
                    for dc in range(4, NDC):
                        nc.tensor.matmul(
                            ps[:], w_sb[:, dc, :],
                            xt_sb[:, b, dc, t * QT_W:(t + 1) * QT_W],
                            start=False, stop=(dc == NDC - 1),
                            skip_group_check=True)
                    nc.vector.tensor_tensor(
                        dst[:, t * QT_W:(t + 1) * QT_W], ps[:],
                        bias[:].to_broadcast([P, QT_W]), ALU.add)

                out += [p1, p2]
            return out

        def v_pieces(b, g):
            """V projection for s-blocks 2g and 2g+1 of batch b: 2 pieces."""
            out = []
            for i in range(2):
                sb = 2 * g + i

                def p(b=b, sb=sb):
                    ps = mm_ps.tile([P, QT_W], f32, tag="mm", name="vps")
                    for dc in range(NDC):
                        nc.tensor.matmul(
                            ps[:, 0:HPC * DH],
                            xt_sb[:, b, dc, sb * P:(sb + 1) * P],
                            wv_sb[:, dc, :],
                            start=(dc == 0), stop=(dc == NDC - 1),
                            skip_group_check=True)
                    nc.vector.tensor_copy(
                        out=vv[b][:, sb, :, 0:DH],
                        in_=ps[:, 0:HPC * DH].rearrange(
                            "p (h e) -> p h e", h=HPC, e=DH))

                out.append(p)
            return out

        oq_rot = [0]

        def oproj_pieces(b, t, zt_sb):
            """Output projection for q-tile t: 8 pieces (c-block halves)."""
            out = []
            for c in range(QT_W // P):
                hold = {}
                for half in range(2):
                    def p(b=b, t=t, c=c, half=half, zt_sb=zt_sb, hold=hold):
                        if half == 0:
                            hold[0] = o_pool.tile([P, D], bf16, tag="o",
                                                  name="osb")
                        o_sb = hold[0]
                        ops = mm_ps.tile([P, QT_W], f32, tag="mm", name="ops")
                        nc.tensor.matmul(
                            ops[:], zt_sb[:, c * P:(c + 1) * P],
                            wo_sb[:, half * 512:(half + 1) * 512],
                            start=True, stop=True)
                        nc.vector.tensor_copy(
                            out=o_sb[:, half * 512:(half + 1) * 512],
                            in_=ops[:])
                        if half == 1:
                            row0 = t * QT_W + c * P
                            q = dmaq[oq_rot[0] % 2]
                            oq_rot[0] += 1
                            q.dma_start(out_d[b, row0:row0 + P, :], o_sb[:])
                    out.append(p)
            return out

        # Attention stages total 80 (+16 trailing-AV points); spread the
        # backlog evenly over the REMAINING stages so late units (batch 1,
        # which has no projection work left) still get PE filler — otherwise
        # the PE micro-idles every stage and the HAM clock gate re-throttles
        # the whole tail of the kernel to 1.2 GHz.
        TOTAL_STAGES = 2 * sum(4 * t + 4 + 2 for t in range(NQT))
        stage_no = [0]

        def pull_adaptive():
            stage_no[0] += 1
            remaining = max(TOTAL_STAGES - stage_no[0], 1)
            n = min(3, -(-len(backlog) // remaining))
            for _ in range(n):
                if backlog:
                    _, piece = backlog.popleft()
                    piece()

        def drain(due):
            while backlog and backlog[0][0] <= due:
                _, piece = backlog.popleft()
                piece()

        # ---------- attention ----------
        def attn_unit(b, t, zt_sb, fin_prev):
            """Scores + softmax + AV for both heads of one (batch, q-tile).

            The two heads' score matmuls go to PE row strips 0:64 / 64:128
            (tile_position) and run concurrently. Software-pipelined by 2
            stages so the exp latency never blocks the score matmuls; filler
            pieces are pulled between stages to keep the FIFO tensor queue
            fed while AV waits on exp."""
            qt_sb, kt_sb, v_sb = qt[b], kt[b], vv[b]
            nkb = 4 * t + 4
            DEPTH = 2
            zps = [z_ps.tile([P, QT_W], f32, tag="z", name=f"zps{h}")
                   for h in range(HPC)]
            pending = []

            def emit_scores(kb):
                j = kb - 4 * t  # >=0 -> diagonal-region block
                width = QT_W - P * j if j >= 0 else QT_W
                qoff = P * j if j >= 0 else 0
                sps = []
                pts = []
                for h in range(HPC):
                    sp = st_ps.tile([P, QT_W], f32, tag="st", name=f"sps{h}")
                    nc.tensor.matmul(
                        sp[:, 0:width],
                        kt_sb[h * DH:(h + 1) * DH, kb * P:(kb + 1) * P],
                        qt_sb[h * DH:(h + 1) * DH,
                              t * QT_W + qoff:(t + 1) * QT_W],
                        start=True, stop=True, tile_position=(h * DH, 0))
                    sps.append(sp)
                for h in range(HPC):
                    pt = pt_pool.tile([P, QT_W], bf16, tag="pt", name=f"pt{h}")
                    nc.scalar.activation(pt[:, 0:width], sps[h][:, 0:width],
                                         AF.Exp)
                    if j >= 0:
                        nc.vector.tensor_tensor(
                            pt[:, 0:P], pt[:, 0:P], msk_sb[:], ALU.mult)
                    pts.append(pt)
                return (kb, pts, width, qoff)

            def emit_av(kb, pts, width, qoff):
                for h in range(HPC):
                    nc.tensor.matmul(
                        zps[h][0:DH + 1, qoff:QT_W],
                        v_sb[:, kb, h, :],
                        pts[h][:, 0:width],
                        start=(kb == 0), stop=(kb == nkb - 1),
                        skip_group_check=True)

            for kb in range(nkb):
                pending.append(emit_scores(kb))
                pull_adaptive()
                if kb < 3:
                    # extra filler at unit start: the previous unit's
                    # normalize chain (rowsum copy -> bcast -> recip -> mult)
                    # still holds the Z' banks our first AVs need.
                    pull_adaptive()
                if kb == 1 and fin_prev is not None:
                    fin_prev()
                if len(pending) > DEPTH:
                    emit_av(*pending.pop(0))
            for item in pending:
                emit_av(*item)
                pull_adaptive()

            # Copy Z' (+ rowsum row) out of PSUM immediately: this is all the
            # next unit's first AVs wait on, and it needs no PE work.
            zraw = [sm_pool.tile([DH + 1, QT_W], bf16, tag=f"zraw{h}",
                                 name=f"zraw{h}") for h in range(HPC)]
            for h in range(HPC):
                nc.vector.tensor_copy(out=zraw[h][:], in_=zps[h][0:DH + 1, :])

            # normalize (DEFERRED: emitted during the NEXT unit's early
            # stages so it is never on the PE critical path, and operates
            # purely on SBUF): Z_h = Z'_h * (1/rowsum_h). Broadcast each
            # head's rowsum row over 64 partitions with a K=65 selector
            # matmul, then one reciprocal and two multiplies.
            def finalize():
                for h in range(HPC):
                    rbps = mm_ps.tile([DH, QT_W], f32, tag="mm",
                                      name="rbps")
                    nc.tensor.matmul(rbps[:], sel_sb[:], zraw[h][:],
                                     start=True, stop=True)
                    rc_sb = sm_pool.tile([DH, QT_W], f32, tag="rc",
                                         name="rc")
                    nc.vector.reciprocal_approx_fast(
                        out=rc_sb[:], in_=rbps[:])
                    nc.vector.tensor_tensor(
                        zt_sb[h * DH:(h + 1) * DH, :], zraw[h][0:DH, :],
                        rc_sb[:], ALU.mult)
                for piece in oproj_pieces(b, t, zt_sb):
                    backlog.append((99, piece))

            return finalize

        # ---------- schedule ----------
        # eager: everything unit (0,0) needs
        for piece in qk_pieces(0, 0) + v_pieces(0, 0) + v_pieces(0, 1):
            piece()

        # backlog: per-unit prerequisites (due = unit index), pulled as
        # filler during earlier units and force-drained at unit entry.
        for t in range(1, NQT):
            for piece in qk_pieces(0, t) + v_pieces(0, 2 * t) \
                    + v_pieces(0, 2 * t + 1):
                backlog.append((t, piece))
        for t in range(NQT):
            for piece in qk_pieces(1, t) + v_pieces(1, 2 * t) \
                    + v_pieces(1, 2 * t + 1):
                backlog.append((4 + t, piece))

        unit_idx = 0
        fin_prev = None
        for b in range(B):
            for t in range(NQT):
                drain(unit_idx)
                zt_sb = zt_pool.tile([P, QT_W], bf16, tag="zt", name="zt")
                fin_prev = attn_unit(b, t, zt_sb, fin_prev)
                unit_idx += 1
        fin_prev()
        drain(99)

    nc.compile()
    return nc


def _prep_in_maps(inputs):
    x = np.asarray(inputs["x"], dtype=np.float32)
    xT = np.ascontiguousarray(x.transpose(0, 2, 1)).astype(BF16)  # [B, D, S]
    W_Q = np.asarray(inputs["W_Q"], dtype=np.float32)
    W_K = np.asarray(inputs["W_K"], dtype=np.float32)
    W_V = np.asarray(inputs["W_V"], dtype=np.float32)
    W_O = np.asarray(inputs["W_O"], dtype=np.float32)
    b_Q = np.asarray(inputs["b_Q"], dtype=np.float32)
    b_K = np.asarray(inputs["b_K"], dtype=np.float32)
    scale = 1.0 / np.sqrt(DH)
    msk01 = np.where(np.arange(P)[:, None] <= np.arange(P)[None, :],
                     np.float32(1.0), np.float32(0.0)).astype(BF16)
    in_maps = []
    for c in range(NCORES):
        hs = [HPC * c + i for i in range(HPC)]
        wq = np.concatenate([W_Q[h] for h in hs], axis=1) * scale
        wk = np.concatenate([W_K[h] for h in hs], axis=1)
        wv = np.concatenate([W_V[h] for h in hs], axis=1)
        wo = np.concatenate([W_O[h] for h in hs], axis=0)
        bq = np.concatenate([b_Q[h] for h in hs])[:, None] * scale
        bk = np.concatenate([b_K[h] for h in hs])[:, None]
        in_maps.append({
            "xT": xT,
            "wq": np.ascontiguousarray(wq).astype(BF16),
            "wk": np.ascontiguousarray(wk).astype(BF16),
            "wv": np.ascontiguousarray(wv).astype(BF16),
            "wo": np.ascontiguousarray(wo).astype(BF16),
            "bq": bq.astype(np.float32),
            "bk": bk.astype(np.float32),
            "msk": msk01,
        })
    return in_maps


def _run(inputs, trace=False, trace_cores=None):
    from concourse.bass_utils import run_bass_kernel_spmd

    if "nc" not in _CACHE:
        _CACHE["nc"] = _build_nc()
    nc = _CACHE["nc"]
    in_maps = _prep_in_maps(inputs)
    res = run_bass_kernel_spmd(
        nc, in_maps, core_ids=list(range(NCORES)),
        trace=trace, trace_cores=trace_cores)

    out = np.zeros((B, S, D), dtype=np.float32)
    for c in range(NCORES):
        out += res.results[c]["out"].astype(np.float32)
    # exact host fold of the zero-pattern-sum bias terms:
    # z includes +b_V per head -> out += sum_h b_V[h] @ W_O[h]; plus b_O.
    b_V = np.asarray(inputs["b_V"], dtype=np.float32)
    W_O = np.asarray(inputs["W_O"], dtype=np.float32)
    b_O = np.asarray(inputs["b_O"], dtype=np.float32)
    out += np.einsum("he,hed->d", b_V, W_O) + b_O

    residual = np.asarray(inputs["residual"], dtype=np.float32)
    return (residual, out), res


def kernel(**inputs):
    (residual, out), _ = _run(inputs, trace=False)
    return residual, out


# revision 31
# speedup vs baseline: 1.0351x; 1.0351x over previous
"""Multi-head causal attention (B=2,S=2048,D=1024,H=16,DH=64) on 8 TRN2 cores.

Sharding: 2 heads per core (tensor parallel). Each core computes QKV for its
2 heads from the full x, causal attention, and its partial of the output
projection [B,S,D]. The host sums the 8 partials (the W_O head-sum).

On-device layouts (matmul contracts over the partition dim):
  QT/KT  [2*DH=128 part, S]   (heads stacked on partitions; 1/sqrt(DH) folded into W_Q)
  V      [S part (128-blocks), heads, DH+1]  (ones column -> softmax row-sums for free)
  S^T    [k 128 part, q 512]  per (k-block, q-tile); above-diagonal blocks skipped
  Z'^T   [DH+1 part, q 512]   accumulated over k-blocks; row DH = exp row-sum
  out    partial [B,S,D] bf16, summed across cores on host

Key scheduling ideas:
  - The two heads' score matmuls are emitted back-to-back with explicit
    tile_position (0,0)/(64,0) so they run CONCURRENTLY in the two 64-row
    strips of the PE array (K=DH=64 each).
  - Causal mask is applied as a 0/1 bf16 multiply on the exp output (DVE,
    cheap) instead of a -1e5 fp32 add on the scores PSUM.
  - All projection / output work is cut into small "filler pieces" that are
    interleaved at per-stage granularity into the attention pipeline; the
    tensor queue is FIFO, so PE work must sit between a score matmul and the
    exp-dependent AV matmul to cover the softmax latency.
  - x^T is DMAed s-tile-major across 4 queues so attention can start early;
    out DMA rotates queues so the tail drains while compute continues.
"""

import os
import sys
from collections import deque

import numpy as np

if "/opt/trn_rl_repo" not in sys.path:
    sys.path.insert(0, "/opt/trn_rl_repo")

import ml_dtypes

B, S, D, H, DH = 2, 2048, 1024, 16, 64
NCORES = 8
HPC = H // NCORES          # heads per core
P = 128
QT_W = 512                 # q-tile width
NQT = S // QT_W            # 4 q-tiles
NKB = S // P               # 16 k-blocks
NDC = D // P               # 8 contraction chunks for projections

BF16 = ml_dtypes.bfloat16

_CACHE = {}


def _build_nc(B=B, S=S, D=D, HPC=HPC, DH=DH):
    import concourse.tile as tile
    import concourse.mybir as mybir
    from concourse import bacc
    from contextlib import ExitStack

    QT_W = 512
    NQT = S // QT_W
    NKB = S // P
    NDC = D // P

    f32 = mybir.dt.float32
    bf16 = mybir.dt.bfloat16
    AF = mybir.ActivationFunctionType
    ALU = mybir.AluOpType

    nc = bacc.Bacc("TRN2", target_bir_lowering=False, debug=False,
                   num_devices=NCORES)

    xT = nc.dram_tensor("xT", [B, D, S], bf16, kind="ExternalInput").ap()
    wq_d = nc.dram_tensor("wq", [D, HPC * DH], bf16, kind="ExternalInput").ap()
    wk_d = nc.dram_tensor("wk", [D, HPC * DH], bf16, kind="ExternalInput").ap()
    wv_d = nc.dram_tensor("wv", [D, HPC * DH], bf16, kind="ExternalInput").ap()
    wo_d = nc.dram_tensor("wo", [HPC * DH, D], bf16, kind="ExternalInput").ap()
    bq_d = nc.dram_tensor("bq", [HPC * DH, 1], f32, kind="ExternalInput").ap()
    bk_d = nc.dram_tensor("bk", [HPC * DH, 1], f32, kind="ExternalInput").ap()
    msk_d = nc.dram_tensor("msk", [P, P], bf16, kind="ExternalInput").ap()
    out_d = nc.dram_tensor("out", [B, S, D], bf16, kind="ExternalOutput").ap()

    with tile.TileContext(nc) as tc, ExitStack() as ctx:
        const = ctx.enter_context(tc.tile_pool(name="const", bufs=1))
        qk_pool = ctx.enter_context(tc.tile_pool(name="qk", bufs=4))
        v_pool = ctx.enter_context(tc.tile_pool(name="v", bufs=2))
        pt_pool = ctx.enter_context(tc.tile_pool(name="pt", bufs=8))
        sm_pool = ctx.enter_context(tc.tile_pool(name="sm", bufs=4))
        zt_pool = ctx.enter_context(tc.tile_pool(name="zt", bufs=4))
        o_pool = ctx.enter_context(tc.tile_pool(name="o", bufs=3))
        # PSUM: 2x2 (paired scores: one [128,1024] tile spans 2 banks)
        # + 2 (Z' per head) + 2 (proj)
        st_ps = ctx.enter_context(tc.tile_pool(name="stps", bufs=2, space="PSUM"))
        z_ps = ctx.enter_context(tc.tile_pool(name="zps", bufs=2, space="PSUM"))
        mm_ps = ctx.enter_context(tc.tile_pool(name="mmps", bufs=2, space="PSUM"))

        dmaq = [nc.sync, nc.gpsimd, nc.scalar]

        # ---- resident constants ----
        # weights first (one per queue, in parallel), then x^T s-tile-major in
        # per-dc chunks so attention on tile 0 can start as early as possible.
        # Per-queue DMA sustains only ~90 GB/s, so what lands first is decided
        # by per-queue byte order, not emission order.
        wq_sb = const.tile([P, NDC, HPC * DH], bf16)
        nc.sync.dma_start(wq_sb[:], wq_d.rearrange("(dc p) m -> p dc m", p=P))
        bq_sb = const.tile([HPC * DH, 1], f32)
        nc.sync.dma_start(bq_sb[:], bq_d[:])
        wk_sb = const.tile([P, NDC, HPC * DH], bf16)
        nc.gpsimd.dma_start(wk_sb[:], wk_d.rearrange("(dc p) m -> p dc m", p=P))
        bk_sb = const.tile([HPC * DH, 1], f32)
        nc.gpsimd.dma_start(bk_sb[:], bk_d[:])
        wv_sb = const.tile([P, NDC, HPC * DH], bf16)
        nc.scalar.dma_start(wv_sb[:], wv_d.rearrange("(dc p) m -> p dc m", p=P))
        msk_sb = const.tile([P, P], bf16)
        nc.scalar.dma_start(msk_sb[:], msk_d[:])
        # rowsum broadcast selector: row DH (the exp row-sum row of the
        # copied-out Z' tile) maps to all 64 output partitions of one head.
        sel_sb = const.tile([DH + 1, DH], bf16)
        nc.vector.memset(sel_sb[:], 0.0)
        nc.vector.memset(sel_sb[DH:DH + 1, :], 1.0)

        # The scalar queue must stay clear of bulk DMA: exp runs there, and a
        # dma_start blocked on ring credits would stall the whole softmax
        # pipeline. Scalar only issues for batch-0 s-tiles 0/1 (early, before
        # any exp); everything later goes to sync/gpsimd.
        xt_sb = const.tile([P, B, NDC, S], bf16)
        wo_sb = const.tile([HPC * DH, D], bf16)
        qrot = 0
        for b in range(B):
            for st in range(NQT):
                s0, s1 = st * QT_W, (st + 1) * QT_W
                nq = 3 if (b == 0 and st < 2) else 2
                for dc in range(NDC):
                    dmaq[qrot % nq].dma_start(
                        xt_sb[:, b, dc, s0:s1],
                        xT[b, dc * P:(dc + 1) * P, s0:s1])
                    qrot += 1
            if b == 0:
                nc.sync.dma_start(wo_sb[:], wo_d[:])

        qt = {}
        kt = {}
        vv = {}
        for b in range(B):
            qt[b] = qk_pool.tile([P, S], bf16, tag="qt", name=f"qt{b}")
            kt[b] = qk_pool.tile([P, S], bf16, tag="qt", name=f"kt{b}")
            vv[b] = v_pool.tile([P, NKB, HPC, DH + 1], bf16, tag="v",
                                name=f"v{b}")
            nc.vector.memset(vv[b][:, :, :, DH:DH + 1], 1.0)

        # ---------- filler pieces ----------
        # Small closures, each ~1-4 matmuls, interleaved between attention
        # pipeline stages. The deque order respects data dependencies; the
        # `due` tag forces a drain before the unit that needs the results.
        backlog = deque()

        def qk_pieces(b, t):
            """Q and K projection for q-tile t of batch b: 4 pieces."""
            out = []
            for w_sb, bias, dst in ((wq_sb, bq_sb, qt[b]),
                                    (wk_sb, bk_sb, kt[b])):
                hold = {}

                def p1(w_sb=w_sb, hold=hold):
                    ps = mm_ps.tile([P, QT_W], f32, tag="mm", name="qkps")
                    for dc in range(4):
                        nc.tensor.matmul(
                            ps[:], w_sb[:, dc, :],
                            xt_sb[:, b, dc, t * QT_W:(t + 1) * QT_W],
                            start=(dc == 0), stop=False,
                            skip_group_check=True)
                    hold[0] = ps

                def p2(w_sb=w_sb, bias=bias, dst=dst, hold=hold):
                    ps = hold[0]
                    for dc in range(4, NDC):
                        nc.tensor.matmul(
                            ps[:], w_sb[:, dc, :],
                            xt_sb[:, b, dc, t * QT_W:(t + 1) * QT_W],
                            start=False, stop=(dc == NDC - 1),
                            skip_group_check=True)
                    nc.vector.tensor_tensor(
                        dst[:, t * QT_W:(t + 1) * QT_W], ps[:],
                        bias[:].to_broadcast([P, QT_W]), ALU.add)

                out += [p1, p2]
            return out

        def v_pieces(b, g):
            """V projection for s-blocks 2g and 2g+1 of batch b: 2 pieces."""
            out = []
            for i in range(2):
                sb = 2 * g + i

                def p(b=b, sb=sb):
                    ps = mm_ps.tile([P, QT_W], f32, tag="mm", name="vps")
                    for dc in range(NDC):
                        nc.tensor.matmul(
                            ps[:, 0:HPC * DH],
                            xt_sb[:, b, dc, sb * P:(sb + 1) * P],
                            wv_sb[:, dc, :],
                            start=(dc == 0), stop=(dc == NDC - 1),
                            skip_group_check=True)
                    nc.vector.tensor_copy(
                        out=vv[b][:, sb, :, 0:DH],
                        in_=ps[:, 0:HPC * DH].rearrange(
                            "p (h e) -> p h e", h=HPC, e=DH))

                out.append(p)
            return out

        oq_rot = [0]

        def oproj_pieces(b, t, zt_sb):
            """Output projection for q-tile t: 8 pieces (c-block halves)."""
            out = []
            for c in range(QT_W // P):
                hold = {}
                for half in range(2):
                    def p(b=b, t=t, c=c, half=half, zt_sb=zt_sb, hold=hold):
                        if half == 0:
                            hold[0] = o_pool.tile([P, D], bf16, tag="o",
                                                  name="osb")
                        o_sb = hold[0]
                        ops = mm_ps.tile([P, QT_W], f32, tag="mm", name="ops")
                        nc.tensor.matmul(
                            ops[:], zt_sb[:, c * P:(c + 1) * P],
                            wo_sb[:, half * 512:(half + 1) * 512],
                            start=True, stop=True)
                        nc.vector.tensor_copy(
                            out=o_sb[:, half * 512:(half + 1) * 512],
                            in_=ops[:])
                        if half == 1:
                            row0 = t * QT_W + c * P
                            q = dmaq[oq_rot[0] % 2]
                            oq_rot[0] += 1
                            q.dma_start(out_d[b, row0:row0 + P, :], o_sb[:])
                    out.append(p)
            return out

        # Attention stages total 80 (+16 trailing-AV points); spread the
        # backlog evenly over the REMAINING stages so late units (batch 1,
        # which has no projection work left) still get PE filler — otherwise
        # the PE micro-idles every stage and the HAM clock gate re-throttles
        # the whole tail of the kernel to 1.2 GHz.
        TOTAL_STAGES = 2 * sum(4 * t + 4 + 2 for t in range(NQT))
        stage_no = [0]

        def pull_adaptive():
            stage_no[0] += 1
            remaining = max(TOTAL_STAGES - stage_no[0], 1)
            n = min(3, -(-len(backlog) // remaining))
            for _ in range(n):
                if backlog:
                    _, piece = backlog.popleft()
                    piece()

        def drain(due):
            keep = []
            while backlog:
                d, piece = backlog.popleft()
                if d <= due:
                    piece()
                else:
                    keep.append((d, piece))
            backlog.extend(keep)

        # ---------- attention ----------
        def attn_unit(b, t, zt_sb, fin_prev):
            """Scores + softmax + AV for both heads of one (batch, q-tile).

            The two heads' score matmuls go to PE row strips 0:64 / 64:128
            (tile_position) and run concurrently. Software-pipelined by 2
            stages so the exp latency never blocks the score matmuls; filler
            pieces are pulled between stages to keep the FIFO tensor queue
            fed while AV waits on exp."""
            qt_sb, kt_sb, v_sb = qt[b], kt[b], vv[b]
            nkb = 4 * t + 4
            DEPTH = 2
            zps = [z_ps.tile([P, QT_W], f32, tag="z", name=f"zps{h}")
                   for h in range(HPC)]
            pending = []

            def emit_scores(kb):
                j = kb - 4 * t  # >=0 -> diagonal-region block
                width = QT_W - P * j if j >= 0 else QT_W
                qoff = P * j if j >= 0 else 0
                # Both heads' scores land in one 2-bank PSUM tile (each
                # matmul within its own bank) so a single exp covers the
                # pair; for diagonal blocks the gap cols hold stale garbage
                # that exp processes but nothing ever reads.
                sps = st_ps.tile([P, 2 * QT_W], f32, tag="st", name="sps")
                for h in range(HPC):
                    nc.tensor.matmul(
                        sps[:, h * QT_W:h * QT_W + width],
                        kt_sb[h * DH:(h + 1) * DH, kb * P:(kb + 1) * P],
                        qt_sb[h * DH:(h + 1) * DH,
                              t * QT_W + qoff:(t + 1) * QT_W],
                        start=True, stop=True, tile_position=(h * DH, 0))
                pt = pt_pool.tile([P, 2 * QT_W], bf16, tag="pt", name="pt")
                nc.scalar.activation(pt[:, 0:QT_W + width],
                                     sps[:, 0:QT_W + width], AF.Exp)
                if j >= 0:
                    for h in range(HPC):
                        nc.vector.tensor_tensor(
                            pt[:, h * QT_W:h * QT_W + P],
                            pt[:, h * QT_W:h * QT_W + P], msk_sb[:],
                            ALU.mult)
                return (kb, pt, width, qoff)

            def emit_av(kb, pt, width, qoff):
                for h in range(HPC):
                    nc.tensor.matmul(
                        zps[h][0:DH + 1, qoff:QT_W],
                        v_sb[:, kb, h, :],
                        pt[:, h * QT_W:h * QT_W + width],
                        start=(kb == 0), stop=(kb == nkb - 1),
                        skip_group_check=True)

            for kb in range(nkb):
                pending.append(emit_scores(kb))
                pull_adaptive()
                if kb < 3:
                    # extra filler at unit start: the previous unit's
                    # normalize chain (rowsum copy -> bcast -> recip -> mult)
                    # still holds the Z' banks our first AVs need.
                    pull_adaptive()
                if kb == 1 and fin_prev is not None:
                    fin_prev()
                if len(pending) > DEPTH:
                    emit_av(*pending.pop(0))
            for item in pending:
                emit_av(*item)
                pull_adaptive()

            # Copy Z' (+ rowsum row) out of PSUM immediately: this is all the
            # next unit's first AVs wait on, and it needs no PE work.
            zraw = [sm_pool.tile([DH + 1, QT_W], bf16, tag=f"zraw{h}",
                                 name=f"zraw{h}") for h in range(HPC)]
            for h in range(HPC):
                nc.vector.tensor_copy(out=zraw[h][:], in_=zps[h][0:DH + 1, :])

            # normalize (DEFERRED: emitted during the NEXT unit's early
            # stages so it is never on the PE critical path, and operates
            # purely on SBUF): Z_h = Z'_h * (1/rowsum_h). Broadcast each
            # head's rowsum row over 64 partitions with a K=65 selector
            # matmul, then one reciprocal and two multiplies.
            def finalize():
                for h in range(HPC):
                    rbps = mm_ps.tile([DH, QT_W], f32, tag="mm",
                                      name="rbps")
                    nc.tensor.matmul(rbps[:], sel_sb[:], zraw[h][:],
                                     start=True, stop=True)
                    rc_sb = sm_pool.tile([DH, QT_W], f32, tag="rc",
                                         name="rc")
                    nc.vector.reciprocal_approx_fast(
                        out=rc_sb[:], in_=rbps[:])
                    nc.vector.tensor_tensor(
                        zt_sb[h * DH:(h + 1) * DH, :], zraw[h][0:DH, :],
                        rc_sb[:], ALU.mult)
                # due = 2 units ahead: force output work to drain steadily
                # instead of piling into a cold end-of-kernel flush
                u = b * NQT + t
                for piece in oproj_pieces(b, t, zt_sb):
                    backlog.append((u + 2, piece))

            return finalize

        # ---------- schedule ----------
        # eager: everything unit (0,0) needs
        for piece in qk_pieces(0, 0) + v_pieces(0, 0) + v_pieces(0, 1):
            piece()

        # backlog: per-unit prerequisites (due = unit index), pulled as
        # filler during earlier units and force-drained at unit entry.
        for t in range(1, NQT):
            for piece in qk_pieces(0, t) + v_pieces(0, 2 * t) \
                    + v_pieces(0, 2 * t + 1):
                backlog.append((t, piece))
        for t in range(NQT):
            for piece in qk_pieces(1, t) + v_pieces(1, 2 * t) \
                    + v_pieces(1, 2 * t + 1):
                backlog.append((4 + t, piece))

        unit_idx = 0
        fin_prev = None
        for b in range(B):
            for t in range(NQT):
                drain(unit_idx)
                zt_sb = zt_pool.tile([P, QT_W], bf16, tag="zt", name="zt")
                fin_prev = attn_unit(b, t, zt_sb, fin_prev)
                unit_idx += 1
        fin_prev()
        drain(99)

    nc.compile()
    return nc


def _prep_in_maps(inputs):
    x = np.asarray(inputs["x"], dtype=np.float32)
    xT = np.ascontiguousarray(x.transpose(0, 2, 1)).astype(BF16)  # [B, D, S]
    W_Q = np.asarray(inputs["W_Q"], dtype=np.float32)
    W_K = np.asarray(inputs["W_K"], dtype=np.float32)
    W_V = np.asarray(inputs["W_V"], dtype=np.float32)
    W_O = np.asarray(inputs["W_O"], dtype=np.float32)
    b_Q = np.asarray(inputs["b_Q"], dtype=np.float32)
    b_K = np.asarray(inputs["b_K"], dtype=np.float32)
    scale = 1.0 / np.sqrt(DH)
    msk01 = np.where(np.arange(P)[:, None] <= np.arange(P)[None, :],
                     np.float32(1.0), np.float32(0.0)).astype(BF16)
    in_maps = []
    for c in range(NCORES):
        hs = [HPC * c + i for i in range(HPC)]
        wq = np.concatenate([W_Q[h] for h in hs], axis=1) * scale
        wk = np.concatenate([W_K[h] for h in hs], axis=1)
        wv = np.concatenate([W_V[h] for h in hs], axis=1)
        wo = np.concatenate([W_O[h] for h in hs], axis=0)
        bq = np.concatenate([b_Q[h] for h in hs])[:, None] * scale
        bk = np.concatenate([b_K[h] for h in hs])[:, None]
        in_maps.append({
            "xT": xT,
            "wq": np.ascontiguousarray(wq).astype(BF16),
            "wk": np.ascontiguousarray(wk).astype(BF16),
            "wv": np.ascontiguousarray(wv).astype(BF16),
            "wo": np.ascontiguousarray(wo).astype(BF16),
            "bq": bq.astype(np.float32),
            "bk": bk.astype(np.float32),
            "msk": msk01,
        })
    return in_maps


def _run(inputs, trace=False, trace_cores=None):
    from concourse.bass_utils import run_bass_kernel_spmd

    if "nc" not in _CACHE:
        _CACHE["nc"] = _build_nc()
    nc = _CACHE["nc"]
    in_maps = _prep_in_maps(inputs)
    res = run_bass_kernel_spmd(
        nc, in_maps, core_ids=list(range(NCORES)),
        trace=trace, trace_cores=trace_cores)

    out = np.zeros((B, S, D), dtype=np.float32)
    for c in range(NCORES):
        out += res.results[c]["out"].astype(np.float32)
    # exact host fold of the zero-pattern-sum bias terms:
    # z includes +b_V per head -> out += sum_h b_V[h] @ W_O[h]; plus b_O.
    b_V = np.asarray(inputs["b_V"], dtype=np.float32)
    W_O = np.asarray(inputs["W_O"], dtype=np.float32)
    b_O = np.asarray(inputs["b_O"], dtype=np.float32)
    out += np.einsum("he,hed->d", b_V, W_O) + b_O

    residual = np.asarray(inputs["residual"], dtype=np.float32)
    return (residual, out), res


def kernel(**inputs):
    (residual, out), _ = _run(inputs, trace=False)
    return residual, out
